# revision 21
# baseline (speedup 1.0000x reference)
"""Multi-head attention forward on 8 Trainium2 NeuronCores (Bass/Tile).

Problem: B=4, S=2048, D_MODEL=1024, H=16, d_k=d_v=64, key-padding mask.
  q = Q@Wq+bq; k = K@Wk+bk; v = V@Wv+bv   (per-head d=64)
  out = softmax(q k^T / sqrt(d) + mask) v      -> [B, S, H*d]

Sharding (hybrid batch x heads over 8 cores): core c handles batch b=c//2
and head-half hh=c%2 (8 heads, output columns hh*512..hh*512+512).

Fully-fused pipeline: the ScalarE exp stream (~160us, the hard floor) starts
~40us in and stays saturated, instead of idling through a separate
projection phase (the 310us two-phase baseline).

  - All projection-path data is bf16: fp32 transposes are two-pass (~256ns)
    and fp32r LDWEIGHTS (~395ns) never hide under `--enable-ldw-opt=false`;
    bf16 single-pass + cheap loads.
  - Eager K / Q(j=0) paths: hwdge fp32 DMA + DVE cast (software-DGE cast
    DMA has a slow cold start), PSUM->SBUF copies on the then-idle ScalarE.
  - Deferred V / Q(j=1) paths stream through gpsimd cast-DMA (warm by then)
    and are drained as cost-bounded "fillers" between the per-m score/exp
    slots, as are the AV chains and output tails.  Budget discipline keeps
    the next m's scores from being delayed past exp-ready (the in-order PE
    queue otherwise converts filler overshoot into ACT bubbles).
  - Scores matmuls (64-deep contraction) are emitted as adjacent pairs on
    opposite PE row groups (partitions 0:64/64:128) which run concurrently
    (measured 1.97x).
  - exp: expS = exp(S^T*scale + mask_bias[partition]); masked keys
    underflow to exactly 0 (scores are O(5), no max subtraction).
  - AV: U^T[65, jj*512] += v_aug_h[m]^T @ expS[m] in a single rotating PSUM
    bank per (hq,jj) chain; row 64 = softmax denominator (ones column).
    av(stage s) chains drain at the start of stage s+1.
  - PSUM banks (8): transposes/out-transposes (2) + projections (1) +
    scores (4) + AV accumulator (1).
"""

from collections import deque

import numpy as np

import concourse.bass as bass
import concourse.mybir as mybir
import concourse.tile as tile
from concourse import bacc
from concourse.bass_utils import run_bass_kernel_spmd

B, S, D, H, DK = 4, 2048, 1024, 16, 64
SK_MIN = 512
OC = 512
HC = 8
P = 128
NB = 512
JB = 1024
SCALE = 1.0 / np.sqrt(float(DK))
NEG = -1.0e9

F32 = mybir.dt.float32
F32R = mybir.dt.float32r
BF16 = mybir.dt.bfloat16

TRACE = False
_CACHE = {}


def _build(SK):
    nc = bacc.Bacc("TRN2", target_bir_lowering=False, debug=False)

    xq = nc.dram_tensor("xq", [S, D], F32, kind="ExternalInput").ap()
    xk = nc.dram_tensor("xk", [SK, D], F32, kind="ExternalInput").ap()
    xv = nc.dram_tensor("xv", [SK, D], F32, kind="ExternalInput").ap()
    wq = nc.dram_tensor("wq", [D, OC], F32, kind="ExternalInput").ap()
    wk = nc.dram_tensor("wk", [D, OC], F32, kind="ExternalInput").ap()
    wv = nc.dram_tensor("wv", [D, OC], F32, kind="ExternalInput").ap()
    bq = nc.dram_tensor("bq", [OC], F32, kind="ExternalInput").ap()
    bk = nc.dram_tensor("bk", [OC], F32, kind="ExternalInput").ap()
    bv = nc.dram_tensor("bv", [OC], F32, kind="ExternalInput").ap()
    mb = nc.dram_tensor("mb", [SK], F32, kind="ExternalInput").ap()
    idin = nc.dram_tensor("idin", [P, P], F32, kind="ExternalInput").ap()
    out = nc.dram_tensor("out", [S, OC], F32, kind="ExternalOutput").ap()

    SKC = SK // P
    DC = D // P
    MC = OC // P
    MS = SKC

    with tile.TileContext(nc) as tc:
        with (
            tc.tile_pool(name="consts", bufs=1) as consts,
            tc.tile_pool(name="persist", bufs=1) as persist,
            tc.tile_pool(name="wpool", bufs=2) as wpool,
            tc.tile_pool(name="xin", bufs=2) as xin,
            tc.tile_pool(name="xbf", bufs=3) as xbf,
            tc.tile_pool(name="xtr", bufs=2) as xtr,
            tc.tile_pool(name="tr_ps", bufs=2, space="PSUM") as tr_ps,
            tc.tile_pool(name="pj_ps", bufs=1, space="PSUM") as pj_ps,
            tc.tile_pool(name="s_ps", bufs=2, space="PSUM") as s_ps,
            tc.tile_pool(name="u_ps", bufs=1, space="PSUM") as u_ps,
            tc.tile_pool(name="expp", bufs=4 * (SK // P)) as expp,
            tc.tile_pool(name="stagep", bufs=2) as stagep,
            tc.tile_pool(name="outp", bufs=2) as outp,
        ):
            # ---------------- constants ----------------
            ident = consts.tile([P, P], F32)
            nc.sync.dma_start(ident[:], idin[:])
            ident_bf = consts.tile([P, P], BF16)
            nc.vector.tensor_copy(ident_bf[:], ident[:])
            mb_sb = consts.tile([P, SKC], F32)
            nc.gpsimd.dma_start(mb_sb[:], mb.rearrange("(m p) -> p m", p=P))
            bias_sb = consts.tile([P, 3, MC], F32)
            nc.gpsimd.dma_start(bias_sb[:, 0, :], bq.rearrange("(m p) -> p m", p=P))
            nc.gpsimd.dma_start(bias_sb[:, 1, :], bk.rearrange("(m p) -> p m", p=P))
            bv_bc = consts.tile([P, OC], F32)
            nc.gpsimd.dma_start(bv_bc[:], bv.partition_broadcast(P))
            ones_sb = consts.tile([P, HC], BF16)
            nc.vector.memset(ones_sb[:], 1.0)
            warm = consts.tile([P, 1], F32)
            warm_in = consts.tile([P, 1], F32)
            nc.vector.memset(warm_in[:], 0.0)
            nc.scalar.activation(warm[:], warm_in[:],
                                 mybir.ActivationFunctionType.Exp)

            qT = persist.tile([P, MC, S], BF16)
            kT = persist.tile([P, MC, SK], BF16)
            v_aug = persist.tile([P, SKC, HC, DK + 1], BF16)

            def load_w(w_in, nm):
                w_sb = wpool.tile([P, DC, NB], BF16, tag="w", name=f"w_{nm}")
                nc.gpsimd.dma_start(w_sb[:], w_in.rearrange("(d p) o -> p d o", p=P))
                return w_sb

            # ---------------- projection machinery ----------------
            def x_emitters(x_in, SX, nm, make_chains, eager):
                """(cost_us, fn) list; transposes of block i+1 zipped 1:1
                between the projection chains of block i."""
                blocks = [(o, min(NB, SX - o)) for o in range(0, SX, NB)]

                def tr_chunk(xT_blk, off, si):
                    def fn(xT_blk=xT_blk, off=off, si=si):
                        sc = off // P + si
                        if eager:
                            x_f = xin.tile([P, D], F32, tag="xf",
                                           name=f"xf_{nm}_{sc}")
                            nc.sync.dma_start(x_f[:], x_in[sc * P:(sc + 1) * P, :])
                            x_sb = xbf.tile([P, D], BF16, tag="xb",
                                            name=f"xb_{nm}_{sc}")
                            nc.vector.tensor_copy(x_sb[:], x_f[:])
                        else:
                            x_sb = xbf.tile([P, D], BF16, tag="xb",
                                            name=f"xb_{nm}_{sc}")
                            nc.gpsimd.dma_start(x_sb[:],
                                                x_in[sc * P:(sc + 1) * P, :])
                        tp = tr_ps.tile([P, DC, P], BF16, tag="tr",
                                        name=f"tr_{nm}_{sc}")
                        for dc in range(DC):
                            nc.tensor.transpose(
                                tp[:, dc, :],
                                x_sb[:, dc * P:(dc + 1) * P],
                                ident_bf[:],
                            )
                        dst = xT_blk[:, :, si * P:(si + 1) * P]
                        if eager:
                            nc.scalar.activation(
                                dst, tp[:], mybir.ActivationFunctionType.Copy)
                        else:
                            nc.vector.tensor_copy(dst, tp[:])
                    return (1.0, fn)

                ems = []
                prev = None
                for off, bw in blocks:
                    xT_blk = xtr.tile([P, DC, NB], BF16, tag="xT",
                                      name=f"xT_{nm}_{off}")
                    trs = [tr_chunk(xT_blk, off, si) for si in range(bw // P)]
                    chains = make_chains(*prev) if prev else []
                    ti = ci = 0
                    while ti < len(trs) or ci < len(chains):
                        if ti < len(trs):
                            ems.append(trs[ti]); ti += 1
                        if ci < len(chains):
                            ems.append(chains[ci]); ci += 1
                    prev = (xT_blk, off, bw)
                ems.extend(make_chains(*prev))
                return ems

            def qk_chains(w_sb, dstT, bias_col, nm, shift=0):
                def make(xT_blk, off, bw):
                    chains = []
                    for mc in range(MC):
                        def fn(xT_blk=xT_blk, off=off, bw=bw, mc=mc):
                            ps = pj_ps.tile([P, NB], F32, tag="pj",
                                            name=f"pj_{nm}_{off}_{mc}")
                            for dc in range(DC):
                                nc.tensor.matmul(
                                    ps[:, 0:bw],
                                    w_sb[:, dc, mc * P:(mc + 1) * P],
                                    xT_blk[:, dc, 0:bw],
                                    start=(dc == 0),
                                    stop=(dc == DC - 1),
                                )
                            nc.vector.tensor_scalar_add(
                                dstT[:, mc, shift + off:shift + off + bw],
                                ps[:, 0:bw],
                                bias_sb[:, bias_col, mc:mc + 1],
                            )
                        chains.append((1.0, fn))
                    return chains
                return make

            def v_chains(w_sb):
                def make(xT_blk, off, bw):
                    chains = []
                    for si in range(bw // P):
                        def fn(xT_blk=xT_blk, off=off, si=si):
                            sc = off // P + si
                            ps = pj_ps.tile([P, NB], F32, tag="pj",
                                            name=f"pjv_{sc}")
                            for dc in range(DC):
                                nc.tensor.matmul(
                                    ps[:],
                                    xT_blk[:, dc, si * P:(si + 1) * P],
                                    w_sb[:, dc, :],
                                    start=(dc == 0),
                                    stop=(dc == DC - 1),
                                )
                            nc.vector.tensor_add(
                                v_aug[:, sc, :, 0:DK],
                                ps[:].rearrange("p (h d) -> p h d", h=HC),
                                bv_bc[:].rearrange("p (h d) -> p h d", h=HC),
                            )
                            nc.vector.tensor_copy(
                                v_aug[:, sc, :, DK:DK + 1], ones_sb[:]
                            )
                        chains.append((1.0, fn))
                    return chains
                return make

            # ---------------- attention machinery ----------------
            exp_tiles = {}
            fillers = deque()
            done_flags = {}

            def drain(budget):
                while fillers and budget > 0:
                    c, fn = fillers[0]
                    if c > budget + 0.7:
                        break
                    fillers.popleft()
                    fn()
                    budget -= c

            def marker(name):
                done_flags[name] = False

                def fn():
                    done_flags[name] = True
                return (0.0, fn)

            def ensure_done(name):
                while not done_flags[name] and fillers:
                    c, fn = fillers.popleft()
                    fn()

            def stage(j, hp, budget=1.4):
                for m in range(MS):
                    sps = []
                    for hq in range(2):
                        s_t = s_ps.tile([P, JB], F32, tag="s",
                                        name=f"s_{hp}_{j}_{m}_{hq}")
                        sps.append(s_t)
                    for jj in range(JB // NB):
                        for hq in range(2):
                            hb = hq * DK
                            nc.tensor.matmul(
                                sps[hq][:, jj * NB:(jj + 1) * NB],
                                kT[hb:hb + DK, hp, m * P:(m + 1) * P],
                                qT[hb:hb + DK, hp,
                                   j * JB + jj * NB:j * JB + (jj + 1) * NB],
                                start=True,
                                stop=True,
                            )
                    for hq in range(2):
                        e = expp.tile([P, JB], BF16, tag="e",
                                      name=f"e_{hp}_{j}_{m}_{hq}")
                        nc.scalar.activation(
                            e[:],
                            sps[hq][:],
                            mybir.ActivationFunctionType.Exp,
                            bias=mb_sb[:, m:m + 1],
                            scale=float(SCALE),
                        )
                        exp_tiles[(j, hp, m, hq)] = e
                    drain(budget)

            def tail_half(j, hp, hq, uT_sb, half):
                h = hp * 2 + hq
                utp = tr_ps.tile([P, 4, DK + 1], F32, tag="tr",
                                 name=f"utp_{hp}_{j}_{hq}_{half}")
                for tt in range(4):
                    nc.tensor.transpose(
                        utp[:, tt, :],
                        uT_sb[:, (half * 4 + tt) * P:(half * 4 + tt + 1) * P],
                        ident[0:DK + 1, 0:DK + 1],
                    )
                u_sb = outp.tile([P, 4, DK + 1], F32, tag="usb",
                                 name=f"usb_{hp}_{j}_{hq}_{half}")
                nc.vector.tensor_copy(u_sb[:], utp[:])
                rec = outp.tile([P, 4, 1], F32, tag="rec",
                                name=f"rec_{hp}_{j}_{hq}_{half}")
                nc.vector.reciprocal(rec[:], u_sb[:, :, DK:DK + 1])
                o_sb = outp.tile([P, 4, DK], F32, tag="osb",
                                 name=f"osb_{hp}_{j}_{hq}_{half}")
                nc.vector.tensor_mul(
                    o_sb[:],
                    u_sb[:, :, 0:DK],
                    rec[:].to_broadcast([P, 4, DK]),
                )
                t0 = j * (JB // P) + half * 4
                nc.sync.dma_start(
                    out.rearrange("(t p) c -> p t c", p=P)[
                        :, t0:t0 + 4, h * DK:(h + 1) * DK
                    ],
                    o_sb[:],
                )

            def av_emitters(j, hp):
                uT = {}
                ems = []
                for hq in range(2):
                    for jj in range(2):
                        def chain(hq=hq, jj=jj):
                            if jj == 0:
                                uT[hq] = stagep.tile(
                                    [DK + 1, JB], F32, tag="uT",
                                    name=f"uT_{hp}_{j}_{hq}")
                            h = hp * 2 + hq
                            u_t = u_ps.tile([DK + 1, NB], F32, tag="u",
                                            name=f"u_{hp}_{j}_{hq}_{jj}")
                            for m in range(MS):
                                e = exp_tiles[(j, hp, m, hq)]
                                nc.tensor.matmul(
                                    u_t[:],
                                    v_aug[:, m, h, :],
                                    e[:, jj * NB:(jj + 1) * NB],
                                    start=(m == 0),
                                    stop=(m == MS - 1),
                                )
                            if jj == 1:
                                for m in range(MS):
                                    exp_tiles.pop((j, hp, m, hq))
                            nc.vector.tensor_copy(
                                uT[hq][:, jj * NB:(jj + 1) * NB], u_t[:])
                        ems.append((1.3, chain))
                    for half in range(2):
                        def tail(hq=hq, half=half):
                            tail_half(j, hp, hq, uT[hq], half)
                        ems.append((0.5, tail))
                return ems

            # ---------------- fused emission ----------------
            w_k = load_w(wk, "k")
            w_q = load_w(wq, "q")

            for c, fn in x_emitters(xk, SK, "k",
                                    qk_chains(w_k, kT, 1, "k"), True):
                fn()
            for c, fn in x_emitters(xq[0:JB, :], JB, "q0",
                                    qk_chains(w_q, qT, 0, "q0"), True):
                fn()

            w_v = load_w(wv, "v")
            fillers.extend(x_emitters(xv, SK, "v", v_chains(w_v), False))
            fillers.append(marker("v_done"))
            stage(0, 0, budget=1.6)
            fillers.extend(x_emitters(xq[JB:S, :], JB, "q1",
                                      qk_chains(w_q, qT, 0, "q1", shift=JB),
                                      False))
            fillers.append(marker("q1_done"))
            stage_list = [(0, 1), (0, 2), (0, 3), (1, 0), (1, 1), (1, 2), (1, 3)]
            prev = (0, 0)
            for (j, hp) in stage_list:
                if (j, hp) == (0, 1):
                    ensure_done("v_done")   # av(0,0) chains read all of v_aug
                if (j, hp) == (1, 0):
                    ensure_done("q1_done")  # j=1 scores read qT[:, :, JB:]
                fillers.extendleft(reversed(av_emitters(*prev)))
                stage(j, hp)
                prev = (j, hp)
            while fillers:
                fillers.popleft()[1]()
            for c, fn in av_emitters(*prev):
                fn()

    nc.compile()
    return nc


def kernel(Q, K, V, mask, Wq, bq, Wk, bk, Wv, bv):
    Q = np.asarray(Q, dtype=np.float32)
    K = np.asarray(K, dtype=np.float32)
    V = np.asarray(V, dtype=np.float32)
    mask = np.asarray(mask)
    Wq = np.asarray(Wq, dtype=np.float32)
    Wk = np.asarray(Wk, dtype=np.float32)
    Wv = np.asarray(Wv, dtype=np.float32)
    bq = np.asarray(bq, dtype=np.float32)
    bk = np.asarray(bk, dtype=np.float32)
    bv = np.asarray(bv, dtype=np.float32)

    max_nk = max(int(np.count_nonzero(mask[b])) for b in range(B))
    SK = max(SK_MIN, -(-max_nk // P) * P)
    if ("nc", SK) not in _CACHE:
        _CACHE[("nc", SK)] = _build(SK)
    nc = _CACHE[("nc", SK)]

    eye = np.eye(P, dtype=np.float32)
    in_maps = []
    for c in range(8):
        b, hh = c // 2, c % 2
        cols = slice(hh * OC, (hh + 1) * OC)
        idx = np.nonzero(mask[b] != 0)[0]
        nk = int(idx.size)
        assert nk <= SK, f"unmasked key count {nk} exceeds compiled capacity {SK}"
        xk_c = np.zeros((SK, D), dtype=np.float32)
        xk_c[:nk] = K[b][idx]
        xv_c = np.zeros((SK, D), dtype=np.float32)
        xv_c[:nk] = V[b][idx]
        mbias = np.full(SK, NEG, dtype=np.float32)
        mbias[:nk] = 0.0
        in_maps.append({
            "xq": np.ascontiguousarray(Q[b]),
            "xk": xk_c,
            "xv": xv_c,
            "wq": np.ascontiguousarray(Wq[:, cols]),
            "wk": np.ascontiguousarray(Wk[:, cols]),
            "wv": np.ascontiguousarray(Wv[:, cols]),
            "bq": np.ascontiguousarray(bq[cols]),
            "bk": np.ascontiguousarray(bk[cols]),
            "bv": np.ascontiguousarray(bv[cols]),
            "mb": mbias.astype(np.float32),
            "idin": eye,
        })

    res = run_bass_kernel_spmd(nc, in_maps, list(range(8)), trace=TRACE)
    _CACHE["last_results"] = res
    _CACHE["exec_time_ns"] = res.exec_time_ns

    full = np.empty((B, S, H * DK), dtype=np.float32)
    for c in range(8):
        b, hh = c // 2, c % 2
        full[b, :, hh * OC:(hh + 1) * OC] = res.results[c]["out"]
    return full
